# revision 20
# baseline (speedup 1.0000x reference)
"""Binarize + packbits kernel for Trainium2 (Bass/Tile), 8-core SPMD.

Reference computation:
    bits = (x[:, None, :] > depth[None, :, None]).astype(uint8)   # [b, 3, i]
    out  = packbits(bits, axis=-1, bitorder="big")                # [b, 3, i/8] uint8

Full shapes: x [4096, 8192] f32, depth [3] f32 -> out [4096, 3, 1024] u8.
Sharding: data-parallel over batch; each of the 8 cores handles 512 rows.

Per-core algorithm (v2):
  For each 128-row tile (4 per core):
    1. compares write {0,1} uint8 bits with the free axis REVERSED, into one
       [128, 3*8192] buffer (planes d=0,1,2).  DVE is_gt runs at 2 elem/cyc;
       ACT Sign(x - t) with a saturating uint8 cast gives the same bits at
       1 elem/cyc, so compares are split DVE/ACT for engine balance.
    2. bit-gather in uint32 words (little-endian packing of the reversed
       bits == big-endian packing of the original order):
         s = (v >> 7) | v ; z = (s >> 14) | s     # nibble in bits 0..3
         c = (z_odd << 4) | z_even                # byte in bits 0..7
       byte 0 of each u32 of c is the packed output byte.
    3. strided byte-0 extract writes the output tile [128, 3*1024] uint8,
       un-reversing each 1024-byte plane.
  Everything synchronized by the Tile framework; a post-pass splits
  multi-semaphore waits (this walrus encodes at most one wait per
  instruction) and fixes integer immediates for bitvec ALU ops.
"""

import numpy as np

import concourse.bass as bass
import concourse.mybir as mybir
from concourse.bass_utils import run_bass_kernel_spmd
from concourse.tile import TileContext

N_CORES = 8
B_FULL = 4096
I_DIM = 8192
D_DIM = 3
J_DIM = I_DIM // 8  # 1024 packed bytes per (b, d)
B_SHARD = B_FULL // N_CORES  # 512
P = 128
N_TILES = B_SHARD // P  # 4

_DT = mybir.dt
_OP = mybir.AluOpType
_LAST_RESULTS = None  # BassKernelResults of the most recent run (for test.py)


def _hoist_multi_waits(nc):
    """walrus codegen here encodes at most ONE semaphore wait per
    instruction ("Too many sync wait commands" otherwise). Hoist every
    multi-wait into a chain of single-wait Drains on the same engine; the
    original instruction then relies on program order behind the drains."""
    for fn in nc.m.functions:
        for bb in fn.blocks:
            new_insts = []
            for inst in bb.instructions:
                si = inst.sync_info
                if si is not None and len(si.on_wait) > 1:
                    for wi, w in enumerate(si.on_wait):
                        new_insts.append(
                            mybir.InstDrain(
                                name=f"{inst.name}-wh{wi}",
                                engine=inst.engine,
                                sync_info=mybir.SyncInfo(
                                    on_wait=[w], on_update=[]
                                ),
                            )
                        )
                    inst.sync_info = mybir.SyncInfo(
                        on_wait=[], on_update=list(si.on_update)
                    )
                new_insts.append(inst)
            bb.instructions[:] = new_insts


_BITVEC_OPS = {
    _OP.logical_shift_left,
    _OP.logical_shift_right,
    _OP.arith_shift_left,
    _OP.arith_shift_right,
    _OP.bitwise_and,
    _OP.bitwise_or,
    _OP.bitwise_xor,
    _OP.bitwise_not,
}


def _fix_int_imms(nc):
    """Bitvec ALU ops require the immediate operand to be an integer typed
    like src/dst; bass lowers python ints as float32 imms. Rewrite them."""
    for fn in nc.m.functions:
        for bb in fn.blocks:
            for inst in bb.instructions:
                op0 = getattr(inst, "op0", None)
                op1 = getattr(inst, "op1", None)
                if op0 not in _BITVEC_OPS and op1 not in _BITVEC_OPS:
                    continue
                ins = list(inst.ins)
                changed = False
                src_dt = None
                for a in ins:
                    if isinstance(a, mybir.ImmediateValue):
                        continue
                    src_dt = a.dtype
                    break
                for i, a in enumerate(ins):
                    if isinstance(a, mybir.ImmediateValue) and src_dt is not None:
                        ins[i] = mybir.ImmediateValue(
                            dtype=src_dt, value=int(a.value)
                        )
                        changed = True
                if changed:
                    inst.ins = ins


def build_bass(depth_vals):
    nc = bass.Bass()
    x = nc.dram_tensor("x", [B_SHARD, I_DIM], _DT.float32, kind="ExternalInput")
    y = nc.dram_tensor("y", [B_SHARD, D_DIM * J_DIM], _DT.uint8, kind="ExternalOutput")

    with TileContext(nc) as tc:
        with (
            tc.tile_pool(name="const", bufs=1) as kpool,
            tc.tile_pool(name="xin", bufs=3) as xpool,
            tc.tile_pool(name="bits", bufs=2) as bpool,
            tc.tile_pool(name="out", bufs=2) as opool,
        ):
            negdep = kpool.tile([P, D_DIM], _DT.float32)
            for d in range(D_DIM):
                nc.vector.memset(negdep[:, d : d + 1], -float(depth_vals[d]))

            for t in range(N_TILES):
                xt = xpool.tile([P, I_DIM], _DT.float32)
                nc.sync.dma_start(out=xt[:], in_=x[t * P : (t + 1) * P, :])

                bits = bpool.tile([P, D_DIM * I_DIM], _DT.uint8)
                # Compares: DVE is_gt runs 2 elem/cyc, ACT Sign 1 elem/cyc.
                # Balance: DVE takes plane 0 plus the front half of plane 1;
                # ACT takes the back half of plane 1 and all of plane 2.
                H = I_DIM // 2
                for d in range(D_DIM):
                    # reversed free axis within plane d
                    plane_rev = bits[:, d * I_DIM : (d + 1) * I_DIM][:, ::-1]
                    dve_n = (I_DIM, H, 0)[d]
                    if dve_n:
                        nc.vector.tensor_scalar(
                            plane_rev[:, :dve_n],
                            xt[:, :dve_n],
                            float(depth_vals[d]),
                            None,
                            _OP.is_gt,
                        )
                    if dve_n < I_DIM:
                        # sign(x - t) -> {-1, 0, 1}; saturating u8 cast -> {0, 1}
                        nc.scalar.activation(
                            plane_rev[:, dve_n:],
                            xt[:, dve_n:],
                            mybir.ActivationFunctionType.Sign,
                            bias=negdep[:, d : d + 1],
                            scale=1.0,
                        )

                v32 = bits[:].bitcast(_DT.uint32)  # [P, 3*2048]
                nc.vector.scalar_tensor_tensor(
                    v32, v32, 7, v32, _OP.logical_shift_right, _OP.bitwise_or
                )
                nc.vector.scalar_tensor_tensor(
                    v32, v32, 14, v32, _OP.logical_shift_right, _OP.bitwise_or
                )
                # after L1+L2, byte 0 of each u32 holds a clean 4-bit nibble;
                # fuse combine + byte extract: out = (nib_odd << 4) | nib_even
                b8 = bits[:].rearrange("p (j eight) -> p j eight", eight=8)
                ot = opool.tile([P, D_DIM * J_DIM], _DT.uint8)
                # un-reverse each 1024-byte plane on the output side
                ot_rev = ot[:].rearrange("p (d j) -> p d j", d=D_DIM)[:, :, ::-1]
                nc.vector.scalar_tensor_tensor(
                    ot_rev,
                    b8[:, :, 4],
                    4,
                    b8[:, :, 0],
                    _OP.logical_shift_left,
                    _OP.bitwise_or,
                )

                nc.sync.dma_start(out=y[t * P : (t + 1) * P, :], in_=ot[:])

    _fix_int_imms(nc)
    _hoist_multi_waits(nc)
    return nc


_NC_CACHE = {}


def _get_nc(depth_vals):
    key = tuple(float(v) for v in depth_vals)
    if key not in _NC_CACHE:
        _NC_CACHE[key] = build_bass(key)
    return _NC_CACHE[key]


def kernel(x, depth):
    global _LAST_RESULTS
    x = np.ascontiguousarray(np.asarray(x), dtype=np.float32)
    depth = np.asarray(depth, dtype=np.float32)
    assert x.shape == (B_FULL, I_DIM), x.shape
    assert depth.shape == (D_DIM,), depth.shape
    shards = x.reshape(N_CORES, B_SHARD, I_DIM)
    in_maps = [{"x": np.ascontiguousarray(shards[i])} for i in range(N_CORES)]
    nc = _get_nc(depth)
    res = run_bass_kernel_spmd(nc, in_maps, core_ids=list(range(N_CORES)))
    _LAST_RESULTS = res
    y = np.stack([r["y"] for r in res.results])  # [8, 512, 3072] u8
    return y.reshape(B_FULL, D_DIM, J_DIM)


# revision 23
# speedup vs baseline: 1.0766x; 1.0766x over previous
"""Binarize + packbits kernel for Trainium2 (Bass/Tile), 8-core SPMD.

Reference computation:
    bits = (x[:, None, :] > depth[None, :, None]).astype(uint8)   # [b, 3, i]
    out  = packbits(bits, axis=-1, bitorder="big")                # [b, 3, i/8] uint8

Full shapes: x [4096, 8192] f32, depth [3] f32 -> out [4096, 3, 1024] u8.
Sharding: data-parallel over batch; each of the 8 cores handles 512 rows.

Per-core algorithm (v2):
  For each 128-row tile (4 per core):
    1. compares write {0,1} uint8 bits with the free axis REVERSED, into one
       [128, 3*8192] buffer (planes d=0,1,2).  DVE is_gt runs at 2 elem/cyc;
       ACT Sign(x - t) with a saturating uint8 cast gives the same bits at
       1 elem/cyc, so compares are split DVE/ACT for engine balance.
    2. bit-gather in uint32 words (little-endian packing of the reversed
       bits == big-endian packing of the original order):
         s = (v >> 7) | v ; z = (s >> 14) | s     # nibble in bits 0..3
         c = (z_odd << 4) | z_even                # byte in bits 0..7
       byte 0 of each u32 of c is the packed output byte.
    3. strided byte-0 extract writes the output tile [128, 3*1024] uint8,
       un-reversing each 1024-byte plane.
  Everything synchronized by the Tile framework; a post-pass splits
  multi-semaphore waits (this walrus encodes at most one wait per
  instruction) and fixes integer immediates for bitvec ALU ops.
"""

import numpy as np

import concourse.bass as bass
import concourse.mybir as mybir
from concourse.bass_utils import run_bass_kernel_spmd
from concourse.tile import TileContext

N_CORES = 8
B_FULL = 4096
I_DIM = 8192
D_DIM = 3
J_DIM = I_DIM // 8  # 1024 packed bytes per (b, d)
B_SHARD = B_FULL // N_CORES  # 512
P = 128
N_TILES = B_SHARD // P  # 4

_DT = mybir.dt
_OP = mybir.AluOpType
_LAST_RESULTS = None  # BassKernelResults of the most recent run (for test.py)


def _hoist_multi_waits(nc):
    """walrus codegen here encodes at most ONE semaphore wait per
    instruction ("Too many sync wait commands" otherwise). Hoist every
    multi-wait into a chain of single-wait Drains on the same engine; the
    original instruction then relies on program order behind the drains."""
    for fn in nc.m.functions:
        for bb in fn.blocks:
            new_insts = []
            for inst in bb.instructions:
                si = inst.sync_info
                if si is not None and len(si.on_wait) > 1:
                    for wi, w in enumerate(si.on_wait):
                        new_insts.append(
                            mybir.InstNoOp(
                                name=f"{inst.name}-wh{wi}",
                                engine=inst.engine,
                                sync_info=mybir.SyncInfo(
                                    on_wait=[w], on_update=[]
                                ),
                            )
                        )
                    inst.sync_info = mybir.SyncInfo(
                        on_wait=[], on_update=list(si.on_update)
                    )
                new_insts.append(inst)
            bb.instructions[:] = new_insts


_BITVEC_OPS = {
    _OP.logical_shift_left,
    _OP.logical_shift_right,
    _OP.arith_shift_left,
    _OP.arith_shift_right,
    _OP.bitwise_and,
    _OP.bitwise_or,
    _OP.bitwise_xor,
    _OP.bitwise_not,
}


def _fix_int_imms(nc):
    """Bitvec ALU ops require the immediate operand to be an integer typed
    like src/dst; bass lowers python ints as float32 imms. Rewrite them."""
    for fn in nc.m.functions:
        for bb in fn.blocks:
            for inst in bb.instructions:
                op0 = getattr(inst, "op0", None)
                op1 = getattr(inst, "op1", None)
                if op0 not in _BITVEC_OPS and op1 not in _BITVEC_OPS:
                    continue
                ins = list(inst.ins)
                changed = False
                src_dt = None
                for a in ins:
                    if isinstance(a, mybir.ImmediateValue):
                        continue
                    src_dt = a.dtype
                    break
                for i, a in enumerate(ins):
                    if isinstance(a, mybir.ImmediateValue) and src_dt is not None:
                        ins[i] = mybir.ImmediateValue(
                            dtype=src_dt, value=int(a.value)
                        )
                        changed = True
                if changed:
                    inst.ins = ins


def build_bass(depth_vals):
    nc = bass.Bass()
    x = nc.dram_tensor("x", [B_SHARD, I_DIM], _DT.float32, kind="ExternalInput")
    y = nc.dram_tensor("y", [B_SHARD, D_DIM * J_DIM], _DT.uint8, kind="ExternalOutput")

    with TileContext(nc) as tc:
        with (
            tc.tile_pool(name="const", bufs=1) as kpool,
            tc.tile_pool(name="xin", bufs=2) as xpool,
            tc.tile_pool(name="bits", bufs=3) as bpool,
            tc.tile_pool(name="out", bufs=2) as opool,
        ):
            negdep = kpool.tile([P, D_DIM], _DT.float32)
            for d in range(D_DIM):
                nc.vector.memset(negdep[:, d : d + 1], -float(depth_vals[d]))

            for t in range(N_TILES):
                xt = xpool.tile([P, I_DIM], _DT.float32)
                nc.sync.dma_start(out=xt[:], in_=x[t * P : (t + 1) * P, :])

                bits = bpool.tile([P, D_DIM * I_DIM], _DT.uint8)
                # Compares: DVE is_gt runs 2 elem/cyc, ACT Sign 1 elem/cyc.
                # Balance: DVE takes plane 0; ACT takes planes 1 and 2.
                for d in range(D_DIM):
                    # reversed free axis within plane d
                    plane_rev = bits[:, d * I_DIM : (d + 1) * I_DIM][:, ::-1]
                    dve_n = (I_DIM, 0, 0)[d]
                    if dve_n:
                        nc.vector.tensor_scalar(
                            plane_rev[:, :dve_n],
                            xt[:, :dve_n],
                            float(depth_vals[d]),
                            None,
                            _OP.is_gt,
                        )
                    if dve_n < I_DIM:
                        # sign(x - t) -> {-1, 0, 1}; saturating u8 cast -> {0, 1}
                        nc.scalar.activation(
                            plane_rev[:, dve_n:],
                            xt[:, dve_n:],
                            mybir.ActivationFunctionType.Sign,
                            bias=negdep[:, d : d + 1],
                            scale=1.0,
                        )

                v32 = bits[:].bitcast(_DT.uint32)  # [P, 3*2048]
                nc.vector.scalar_tensor_tensor(
                    v32, v32, 7, v32, _OP.logical_shift_right, _OP.bitwise_or
                )
                nc.vector.scalar_tensor_tensor(
                    v32, v32, 14, v32, _OP.logical_shift_right, _OP.bitwise_or
                )
                # after L1+L2, byte 0 of each u32 holds a clean 4-bit nibble;
                # fuse combine + byte extract: out = (nib_odd << 4) | nib_even
                b8 = bits[:].rearrange("p (j eight) -> p j eight", eight=8)
                ot = opool.tile([P, D_DIM * J_DIM], _DT.uint8)
                # un-reverse each 1024-byte plane on the output side
                ot_rev = ot[:].rearrange("p (d j) -> p d j", d=D_DIM)[:, :, ::-1]
                nc.vector.scalar_tensor_tensor(
                    ot_rev,
                    b8[:, :, 4],
                    4,
                    b8[:, :, 0],
                    _OP.logical_shift_left,
                    _OP.bitwise_or,
                )

                nc.sync.dma_start(out=y[t * P : (t + 1) * P, :], in_=ot[:])

    _fix_int_imms(nc)
    _hoist_multi_waits(nc)
    return nc


_NC_CACHE = {}


def _get_nc(depth_vals):
    key = tuple(float(v) for v in depth_vals)
    if key not in _NC_CACHE:
        _NC_CACHE[key] = build_bass(key)
    return _NC_CACHE[key]


def kernel(x, depth):
    global _LAST_RESULTS
    x = np.ascontiguousarray(np.asarray(x), dtype=np.float32)
    depth = np.asarray(depth, dtype=np.float32)
    assert x.shape == (B_FULL, I_DIM), x.shape
    assert depth.shape == (D_DIM,), depth.shape
    shards = x.reshape(N_CORES, B_SHARD, I_DIM)
    in_maps = [{"x": np.ascontiguousarray(shards[i])} for i in range(N_CORES)]
    nc = _get_nc(depth)
    res = run_bass_kernel_spmd(nc, in_maps, core_ids=list(range(N_CORES)))
    _LAST_RESULTS = res
    y = np.stack([r["y"] for r in res.results])  # [8, 512, 3072] u8
    return y.reshape(B_FULL, D_DIM, J_DIM)
